# revision 49
# baseline (speedup 1.0000x reference)
"""Causal multi-head attention block (b=8, s=1024, d_model=768, 12 heads x 64)
on 8 TRN2 NeuronCores — batch-parallel: core i computes batch element i.

Self-contained: includes the NTFF-profile-hook shim and the BIR wait-split
workaround for this walrus build (max 1 semaphore wait per instruction).

Per-core plan (bf16 matmuls, fp32 PSUM accumulation):
  A. x arrives bf16 (host-transposed) -> xT tiles [m][128,1024] via HWDGE
     (sync/scalar rings), weights ordered by first use (wv, wq, wk early;
     wo late on the gpsimd SWDGE ring).
  B. QT/KT [hd-blk][128,1024] = W.T @ xT (head-pair packed); V in natural
     [s,hd] layout padded to 65 cols/head with a ones column (rowsum trick).
     Only the half-0 slices + V st0..3 are emitted up front; everything
     else drains through a feeder into PE gaps inside the attention loop.
  C. attention is software-pipelined per q-half(512)/head-pair:
     scores(kt+1) is emitted before pv(kt) so the PE never waits on the
     exp of the current tile; exp on ACT (1/8 scale folded), causal mask
     via gpsimd affine_select on diagonal k-tiles, PV accumulates
     [65,512]x2 in PSUM (row 64 = softmax denominator).
  D. denominators collect into a [6,1024] tile per half; one DVE
     reciprocal_approx_fast + f32r copy per half; per-head broadcast via
     K=1 matmul; normalize into the [hd,q] stack; out-proj + b_O; DMA out.
"""

import os
import sys
import types

import numpy as np

# ---------------------------------------------------------------------------
# environment shims


def _install_ntff_hook():
    try:
        import antenv
        from trn_agent_boot.trn_boot import _ntff_profile_via_ctypes
    except Exception:
        return
    if "antenv.axon_hooks" in sys.modules:
        return
    hook = _ntff_profile_via_ctypes("/opt/axon/libaxon_pjrt.so")
    m = types.ModuleType("antenv.axon_hooks")
    m.set_axon_ntff_profile_hook = lambda h: None
    m.get_axon_ntff_profile_hook = lambda: hook
    sys.modules["antenv.axon_hooks"] = m
    antenv.axon_hooks = m


def _install_waitsplit(max_waits=1):
    """walrus on this build rejects >1 sem wait per instruction; split extras
    onto preceding NoOps (same engine, program order preserved)."""
    import json

    import concourse.bass as bass

    if getattr(bass.Bass, "_waitsplit_installed", False):
        return
    counter = [0]

    def _split(inst):
        si = inst.get("sync_info")
        if not si:
            return [inst]
        waits = si.get("on_wait") or []
        if len(waits) <= max_waits:
            return [inst]
        out = []
        head, rest = waits[:-max_waits], waits[-max_waits:]
        for i in range(0, len(head), max_waits):
            counter[0] += 1
            out.append(
                {
                    "debug": inst.get("debug", 0),
                    "engine": inst["engine"],
                    "ins": [],
                    "name": f"I-waitsplit-{counter[0]}",
                    "opcode": "NoOp",
                    "outs": [],
                    "text_hint": "waitsplit",
                    "sync_info": {
                        "on_update": [],
                        "on_wait": head[i : i + max_waits],
                    },
                }
            )
        si["on_wait"] = rest
        out.append(inst)
        return out

    orig = bass.Bass.to_json_bytes

    def to_json_bytes(self):
        d = json.loads(orig(self))
        changed = False
        for f in d.get("functions", []):
            for bb in f.get("blocks", []):
                new = []
                for inst in bb.get("instructions", []):
                    parts = _split(inst)
                    changed = changed or len(parts) > 1
                    new.extend(parts)
                bb["instructions"] = new
        return json.dumps(d).encode() if changed else orig(self)

    bass.Bass.to_json_bytes = to_json_bytes
    bass.Bass._waitsplit_installed = True


_install_ntff_hook()
_install_waitsplit()

import ml_dtypes  # noqa: E402
import concourse.bass as bass  # noqa: E402
import concourse.mybir as mybir  # noqa: E402
import concourse.tile as tile  # noqa: E402
from concourse.bass_utils import run_bass_kernel_spmd  # noqa: E402

# ---------------------------------------------------------------------------
# problem constants (hardcoded per harness contract)

B, S, D, H, DH = 8, 1024, 768, 12, 64
P = 128
MT = D // P            # 6 tiles over d_model / hd
QC = 256               # q-chunk width
QH = 512               # q-half (pair of chunks)
NKT = S // P           # 8 k-tiles over seq
SCALE = float(1.0 / np.sqrt(DH))
N_CORES = 8

F32 = mybir.dt.float32
F32R = mybir.dt.float32r
BF16 = mybir.dt.bfloat16
MMDT = BF16


def build_nc() -> bass.Bass:
    nc = bass.Bass()
    xT = nc.declare_dram_parameter("xT", [D, S], MMDT, isOutput=False)
    wq = nc.declare_dram_parameter("wq", [D, D], MMDT, isOutput=False)
    wk = nc.declare_dram_parameter("wk", [D, D], MMDT, isOutput=False)
    wv = nc.declare_dram_parameter("wv", [D, D], MMDT, isOutput=False)
    wo = nc.declare_dram_parameter("wo", [D, D], MMDT, isOutput=False)
    bq = nc.declare_dram_parameter("bq", [D], F32, isOutput=False)
    bk = nc.declare_dram_parameter("bk", [D], F32, isOutput=False)
    bv = nc.declare_dram_parameter("bv", [D], F32, isOutput=False)
    bo = nc.declare_dram_parameter("bo", [D], F32, isOutput=False)
    y = nc.declare_dram_parameter("y", [S, D], F32, isOutput=True)

    Exp = mybir.ActivationFunctionType.Exp
    Ident = mybir.ActivationFunctionType.Identity
    mult = mybir.AluOpType.mult
    add = mybir.AluOpType.add
    is_ge = mybir.AluOpType.is_ge

    from collections import deque
    from contextlib import ExitStack

    with ExitStack() as _ctx:
        tc = _ctx.enter_context(tile.TileContext(nc))
        constp = _ctx.enter_context(tc.tile_pool(name="const", bufs=1))
        xtp = _ctx.enter_context(tc.tile_pool(name="xT", bufs=1))
        qtp = _ctx.enter_context(tc.tile_pool(name="qt", bufs=1))
        ktp = _ctx.enter_context(tc.tile_pool(name="kt", bufs=1))
        vpp = _ctx.enter_context(tc.tile_pool(name="vp", bufs=1))
        wtsp = _ctx.enter_context(tc.tile_pool(name="wts", bufs=24))
        expp = _ctx.enter_context(tc.tile_pool(name="expst", bufs=6))
        wsp = _ctx.enter_context(tc.tile_pool(name="wstack", bufs=12))
        outp = _ctx.enter_context(tc.tile_pool(name="outsb", bufs=2))
        nrmp = _ctx.enter_context(tc.tile_pool(name="nrm", bufs=2))
        psflow = _ctx.enter_context(
            tc.tile_pool(name="ps_flow", bufs=2, space="PSUM")
        )
        psacc = _ctx.enter_context(
            tc.tile_pool(name="ps_acc", bufs=2, space="PSUM")
        )
        scpp = _ctx.enter_context(
            tc.tile_pool(name="ps_scp", bufs=2, space="PSUM")
        )

        # ---- input DMAs, ordered by first use --------------------------
        # HWDGE rings: sync (SP) and scalar (ACT); SWDGE (gpsimd) for the
        # late-needed W_O and small rows.
        # consolidated input DMAs: one transfer per (tensor, mt-parity) —
        # partition p of the even tile gets rows {0,2,4}*128+p as three
        # column-blocks (3D source AP), so each ring issues 1 trigger per
        # tensor half instead of 3.
        def load_pair(dram, cols, pool, tag, dt):
            base = dram[:, :]
            tiles = []
            for par, eng in ((0, nc.sync), (1, nc.scalar)):
                t = pool.tile([P, 3 * cols], dt, tag=f"{tag}{par}",
                              bufs=1, name=f"{tag}{par}")
                src = bass.AP(
                    base.tensor, par * P * cols,
                    [[cols, P], [2 * P * cols, 3], [1, cols]],
                )
                eng.dma_start(t[:], src)
                tiles.append(t)
            def sl(mt, a, b):
                return tiles[mt % 2][:, (mt // 2) * cols + a : (mt // 2) * cols + b]
            return sl

        xsl = load_pair(xT, S, xtp, "xe", MMDT)
        wqsl = load_pair(wq, D, wtsp, "wqe", MMDT)
        wksl = load_pair(wk, D, wtsp, "wke", MMDT)
        wvsl = load_pair(wv, D, wtsp, "wve", MMDT)
        bq_t = constp.tile([P, MT], F32, tag="bq")  # col hdb = bias block
        bk_t = constp.tile([P, MT], F32, tag="bk")
        nc.sync.dma_start(bq_t[:], bq.rearrange("(c p) -> p c", p=P))
        nc.scalar.dma_start(bk_t[:], bk.rearrange("(c p) -> p c", p=P))
        bv_stage = constp.tile([1, D], F32, tag="bvstage")
        bo_stage = constp.tile([1, D], F32, tag="bostage")
        nc.gpsimd.dma_start(bv_stage[:], bv.rearrange("(o d) -> o d", o=1))
        nc.gpsimd.dma_start(bo_stage[:], bo.rearrange("(o d) -> o d", o=1))
        wo_t = []
        for par in range(2):
            t = wtsp.tile([P, 3 * D], MMDT, tag=f"woe{par}", bufs=1,
                          name=f"woe{par}")
            src = bass.AP(
                wo[:, :].tensor, par * P * D,
                [[D, P], [2 * P * D, 3], [1, D]],
            )
            nc.gpsimd.dma_start(t[:], src)
            wo_t.append(t)

        def wosl(mt, a, b):
            return wo_t[mt % 2][:, (mt // 2) * D + a : (mt // 2) * D + b]

        # ---- on-chip constants ----------------------------------------
        ones_stage = constp.tile([1, P], F32, tag="onesstage")
        nc.vector.memset(ones_stage[:], 1.0)
        ones_row = constp.tile([1, P], F32R, tag="onesrow")
        nc.vector.tensor_copy(ones_row[:], ones_stage[:])
        ones_col = constp.tile([P, H], F32, tag="onescol")
        nc.vector.memset(ones_col[:], 1.0)
        bv_row = constp.tile([1, D], F32R, tag="bvrow")
        bo_row = constp.tile([1, D], F32R, tag="borow")
        nc.vector.tensor_copy(bv_row[:], bv_stage[:])
        nc.vector.tensor_copy(bo_row[:], bo_stage[:])

        # selector lhsT for the denominator broadcast: one K=33 matmul
        # replicates row 0 over out partitions 0-63 and row 32 over 64-127
        sel2 = constp.tile([33, P], BF16, tag="sel2")
        nc.vector.memset(sel2[:], 0.0)
        nc.vector.memset(sel2[0:1, 0:64], 1.0)
        nc.vector.memset(sel2[32:33, 64:128], 1.0)
        # per-(half,head) softmax denominator rows: sub0 at partition 0,
        # sub1 at partition 32; rows 1-31 zeroed once (0 x garbage = NaN)
        dns = [
            [
                constp.tile([33, QH], BF16, tag=f"dn{pp}_{hp}",
                            name=f"dn{pp}_{hp}")
                for hp in range(MT)
            ]
            for pp in range(2)
        ]
        for pp in range(2):
            for hp in range(MT):
                nc.vector.memset(dns[pp][hp][:], 0.0)

        # broadcast bias rows to all partitions via K=1 outer-product matmul
        # (emitted after the prologue projections so they don't block PE on
        # the bv/bo DMAs — see emission order below)
        bv_b = constp.tile([P, D], F32, tag="bvb")
        bo_b = constp.tile([P, D], F32, tag="bob")

        def emit_bias_broadcasts():
            for row, bcast in ((bv_row, bv_b), (bo_row, bo_b)):
                for c0, c1 in ((0, 512), (512, 768)):
                    bps = psflow.tile([P, 512], F32, tag="ps", name="bps")
                    nc.tensor.matmul(
                        bps[:, : c1 - c0],
                        ones_row[:],
                        row[:, c0:c1],
                        start=True,
                        stop=True,
                        skip_group_check=True,
                    )
                    nc.vector.tensor_copy(bcast[:, c0:c1], bps[:, : c1 - c0])

        # ---- projection generators (feeder work units) -----------------
        qts = [qtp.tile([P, S], MMDT, tag=f"qt{i}", name=f"qt{i}") for i in range(MT)]
        kts = [ktp.tile([P, S], MMDT, tag=f"kt{i}", name=f"kt{i}") for i in range(MT)]
        vps = [
            vpp.tile([P, H * 65], MMDT, tag=f"vp{st}", name=f"vp{st}")
            for st in range(NKT)
        ]

        def proj_qk_gen(wsl, b_t, dst, sc, hdb, on_act):
            s0 = sc * 512
            ps0 = psflow.tile([P, 512], F32, tag="ps", name="pj0")
            for mt in range(MT):
                nc.tensor.matmul(
                    ps0[:], wsl(mt, hdb * P, (hdb + 1) * P),
                    xsl(mt, s0, s0 + 512),
                    start=(mt == 0), stop=(mt == MT - 1),
                    skip_group_check=True,
                )
                if mt == 2:
                    yield
            if on_act:
                # fused bias add on the scalar engine (per-partition bias)
                nc.scalar.activation(
                    dst[hdb][:, s0 : s0 + 512], ps0[:], Ident,
                    bias=b_t[:, hdb : hdb + 1],
                )
            else:
                bsl = b_t[:, hdb : hdb + 1]
                bb = bass.AP(bsl.tensor, bsl.offset, [bsl.ap[0], [0, 512]])
                nc.vector.tensor_tensor(
                    dst[hdb][:, s0 : s0 + 512], ps0[:], bb, op=add
                )
            yield

        def proj_qk_piece(w_t, b_t, dst, sc, hdb, on_act=False):
            for _ in proj_qk_gen(w_t, b_t, dst, sc, hdb, on_act):
                pass

        def proj_v_gen(st, atomic=False):
            vv = vps[st].rearrange("p (h c) -> p h c", c=65)
            nc.vector.tensor_copy(
                vv[:, :, 64:65],
                ones_col.rearrange("p (h c) -> p h c", c=1),
            )
            ps0 = psflow.tile([P, 512], F32, tag="ps", name="pv0")
            ps1 = psflow.tile([P, 512], F32, tag="ps", name="pv1")
            for mt in range(MT):
                lx = xsl(mt, st * P, (st + 1) * P)
                nc.tensor.matmul(
                    ps0[:], lx, wvsl(mt, 0, 512),
                    start=(mt == 0), stop=(mt == MT - 1),
                    skip_group_check=True,
                )
                nc.tensor.matmul(
                    ps1[:, 0:256], lx, wvsl(mt, 512, 768),
                    start=(mt == 0), stop=(mt == MT - 1),
                    skip_group_check=True,
                )
                if not atomic and mt in (1, 3):
                    yield
            bsrc = bv_b.rearrange("p (h c) -> p h c", c=DH)
            nc.vector.tensor_tensor(
                vv[:, 0:8, 0:DH],
                ps0.rearrange("p (h c) -> p h c", c=DH),
                bsrc[:, 0:8, :],
                op=add,
            )
            nc.vector.tensor_tensor(
                vv[:, 8:12, 0:DH],
                ps1[:, 0:256].rearrange("p (h c) -> p h c", c=DH),
                bsrc[:, 8:12, :],
                op=add,
            )
            yield

        def proj_v(st):
            for _ in proj_v_gen(st):
                pass

        def outproj_gen(pp, wstack, sub):
            q0 = pp * QH
            opsa = psflow.tile([P, 512], F32, tag="ps", name="opa_t")
            opsb = psflow.tile([P, 512], F32, tag="ps", name="opb_t")
            for hdt in range(MT):
                lw = wstack[hdt][:, sub * P : (sub + 1) * P]
                nc.tensor.matmul(
                    opsa[:], lw, wosl(hdt, 0, 512),
                    start=(hdt == 0), stop=(hdt == MT - 1),
                    skip_group_check=True,
                )
                nc.tensor.matmul(
                    opsb[:, 0:256], lw, wosl(hdt, 512, 768),
                    start=(hdt == 0), stop=(hdt == MT - 1),
                    skip_group_check=True,
                )
                if hdt in (1, 3):
                    yield
            osb = outp.tile([P, D], F32, tag="osb")
            nc.vector.tensor_tensor(
                osb[:, 0:512], opsa[:], bo_b[:, 0:512], op=add
            )
            nc.vector.tensor_tensor(
                osb[:, 512:768], opsb[:, 0:256], bo_b[:, 512:768], op=add
            )
            nc.sync.dma_start(
                y[q0 + sub * P : q0 + (sub + 1) * P, :], osb[:]
            )
            yield

        def outproj_sub(pp, wstack, sub):
            for _ in outproj_gen(pp, wstack, sub):
                pass

        # half-1 out-projection in two passes: pass A (heads 0-3 + b_O)
        # stages to SBUF and is feedable during the last attention head;
        # pass B (heads 4-5) merges and writes out — the only true tail.
        stA = [
            nrmp.tile([P, D], F32, tag=f"stA{s}", bufs=1, name=f"stA{s}")
            for s in range(4)
        ]

        def op1_passA_gen(wstack, sub):
            opsa = psflow.tile([P, 512], F32, tag="ps", name="opa_t")
            opsb = psflow.tile([P, 512], F32, tag="ps", name="opb_t")
            for hdt in range(4):
                lw = wstack[hdt][:, sub * P : (sub + 1) * P]
                nc.tensor.matmul(
                    opsa[:], lw, wosl(hdt, 0, 512),
                    start=(hdt == 0), stop=(hdt == 3),
                    skip_group_check=True,
                )
                nc.tensor.matmul(
                    opsb[:, 0:256], lw, wosl(hdt, 512, 768),
                    start=(hdt == 0), stop=(hdt == 3),
                    skip_group_check=True,
                )
                if hdt == 1:
                    yield
            nc.vector.tensor_tensor(
                stA[sub][:, 0:512], opsa[:], bo_b[:, 0:512], op=add
            )
            nc.vector.tensor_tensor(
                stA[sub][:, 512:768], opsb[:, 0:256], bo_b[:, 512:768], op=add
            )
            yield

        def op1_passB(wstack, sub):
            opsa = psflow.tile([P, 512], F32, tag="ps", name="opa_t")
            opsb = psflow.tile([P, 512], F32, tag="ps", name="opb_t")
            for hdt in range(4, MT):
                lw = wstack[hdt][:, sub * P : (sub + 1) * P]
                nc.tensor.matmul(
                    opsa[:], lw, wosl(hdt, 0, 512),
                    start=(hdt == 4), stop=(hdt == MT - 1),
                    skip_group_check=True,
                )
                nc.tensor.matmul(
                    opsb[:, 0:256], lw, wosl(hdt, 512, 768),
                    start=(hdt == 4), stop=(hdt == MT - 1),
                    skip_group_check=True,
                )
            osb = outp.tile([P, D], F32, tag="osb")
            nc.vector.tensor_tensor(
                osb[:, 0:512], opsa[:], stA[sub][:, 0:512], op=add
            )
            nc.vector.tensor_tensor(
                osb[:, 512:768], opsb[:, 0:256], stA[sub][:, 512:768], op=add
            )
            nc.sync.dma_start(
                y[QH + sub * P : QH + (sub + 1) * P, :], osb[:]
            )

        def norm_one(pp, hp, wstack):
            """Broadcast the head-pair's two raw denominator rows into one
            [128,512] PSUM tile via a single K=33 selector matmul,
            reciprocal across all 128 partitions at once, then one
            full-width normalize multiply."""
            rb = psflow.tile([P, 512], F32, tag="ps", name="rb")
            nc.tensor.matmul(
                rb[:], sel2[:], dns[pp][hp][:],
                start=True, stop=True, skip_group_check=True,
            )
            rsb = nrmp.tile([P, 512], F32, tag="rsb", name="rsb")
            nc.vector.reciprocal_approx_fast(rsb[:], rb[:])
            nc.vector.tensor_tensor(
                wstack[hp][:], wstack[hp][:], rsb[:], op=mult,
            )

        def norm_burst_gen(pp, wstack):
            for hp in range(MT):
                norm_one(pp, hp, wstack)
                yield

        def norm_one_gen(pp, hp, wstack):
            norm_one(pp, hp, wstack)
            yield

        class Feeder:
            """Doles out deferred emission work in ~2-3-matmul steps so the
            PE stream interleaves finely with attention matmuls."""

            def __init__(self):
                self.q = deque()

            def add(self, tag, gen):
                self.q.append((tag, gen))

            def step(self):
                while self.q:
                    try:
                        next(self.q[0][1])
                        return
                    except StopIteration:
                        self.q.popleft()

            def drain_until(self, tag):
                while any(t == tag for t, _ in self.q):
                    try:
                        next(self.q[0][1])
                    except StopIteration:
                        self.q.popleft()

            def drain(self):
                while self.q:
                    self.step()

        feeder = Feeder()

        def attn_core(pp, hp, wstack, feed):
            q0 = pp * QH
            nkt0 = 4 * pp + 2
            nkt1 = 4 * pp + 4
            pvs = [
                psacc.tile([65, QH], F32, tag="pv", name=f"pv{sub}")
                for sub in range(2)
            ]
            scps = {}
            ests = {}

            def emit_scores(kt):
                c0 = 0 if kt < nkt0 else QC
                scp = scpp.tile([P, 2 * QH], F32, tag="scp", name="scp")
                for sub in range(2):
                    r0 = sub * 64
                    nc.tensor.matmul(
                        scp[:, sub * QH + c0 : (sub + 1) * QH],
                        kts[hp][r0 : r0 + 64, kt * P : (kt + 1) * P],
                        qts[hp][r0 : r0 + 64, q0 + c0 : q0 + QH],
                        start=True,
                        stop=True,
                        tile_position=(r0, 0),
                        skip_group_check=True,
                    )
                scps[kt] = (scp, c0)

            def emit_exp_mask(kt):
                scp, c0 = scps.pop(kt)
                w = QH - c0
                est = expp.tile([P, 2 * QH], MMDT, tag="est", name="est")
                if c0 == 0:
                    nc.scalar.activation(est[:], scp[:], Exp, scale=SCALE)
                else:
                    sin = bass.AP(
                        scp.tensor, scp.offset + c0,
                        [scp.ap[0], [QH, 2], [1, w]],
                    )
                    sout = bass.AP(
                        est.tensor, est.offset + c0,
                        [est.ap[0], [QH, 2], [1, w]],
                    )
                    nc.scalar.activation(sout, sin, Exp, scale=SCALE)
                for sub in range(2):
                    b0 = sub * QH
                    if kt in (4 * pp, 4 * pp + 1):
                        nc.gpsimd.affine_select(
                            est[:, b0 : b0 + QC], est[:, b0 : b0 + QC],
                            pattern=[[1, QC]],
                            compare_op=is_ge, fill=0.0,
                            base=(0 if kt == 4 * pp else -P),
                            channel_multiplier=-1,
                        )
                    if kt in (4 * pp + 2, 4 * pp + 3):
                        nc.gpsimd.affine_select(
                            est[:, b0 + QC : b0 + QH],
                            est[:, b0 + QC : b0 + QH],
                            pattern=[[1, QC]],
                            compare_op=is_ge, fill=0.0,
                            base=(0 if kt == 4 * pp + 2 else -P),
                            channel_multiplier=-1,
                        )
                ests[kt] = (est, c0)

            def emit_pv(kt):
                est, c0 = ests.pop(kt)
                for sub in range(2):
                    h = 2 * hp + sub
                    nc.tensor.matmul(
                        pvs[sub][:, c0:QH],
                        vps[kt][:, h * 65 : (h + 1) * 65],
                        est[:, sub * QH + c0 : (sub + 1) * QH],
                        start=(kt == 0),
                        stop=(kt == nkt1 - 1),
                        skip_group_check=True,
                    )

            emit_scores(0)
            for kt in range(nkt1):
                if kt + 1 < nkt1:
                    emit_scores(kt + 1)
                emit_exp_mask(kt)
                feed()
                emit_pv(kt)
            # stash frees the PV banks: unnormalized rows into wstack (bf16;
            # half-0 casts ride the scalar engine, which has slack there),
            # denominator rows into partitions 0/32 of the dn tile
            for sub in range(2):
                r0 = sub * 64
                if pp == 0:
                    nc.scalar.activation(
                        wstack[hp][r0 : r0 + 64, :], pvs[sub][0:64, :], Ident
                    )
                else:
                    nc.vector.tensor_copy(
                        wstack[hp][r0 : r0 + 64, :], pvs[sub][0:64, :]
                    )
                nc.vector.tensor_copy(
                    dns[pp][hp][32 * sub : 32 * sub + 1, :],
                    pvs[sub][64:65, :],
                )

        # ---- emission order -------------------------------------------
        wstack0 = [
            wsp.tile([P, QH], MMDT, tag="ws", name=f"ws0_{i}")
            for i in range(MT)
        ]
        wstack1 = [
            wsp.tile([P, QH], MMDT, tag="ws", name=f"ws1_{i}")
            for i in range(MT)
        ]

        # prologue: bias broadcasts (bv_b must be written before proj_v's
        # epilogue reads it), Q/K half-0 hp0 (wq/wk arrive first on the
        # HWDGE rings), V st0; V st1-3 are atomic feeder chunks consumed
        # inside hp0's kt loop just ahead of their pv(kt) consumers.
        emit_bias_broadcasts()
        proj_qk_piece(wqsl, bq_t, qts, 0, 0, on_act=True)
        proj_qk_piece(wksl, bk_t, kts, 0, 0, on_act=True)
        proj_v(0)

        for st in range(1, 4):
            feeder.add(("v", st), proj_v_gen(st, atomic=True))
        for hp in range(1, MT):
            feeder.add(("q0", hp),
                       proj_qk_gen(wqsl, bq_t, qts, 0, hp, True))
            feeder.add(("k0", hp),
                       proj_qk_gen(wksl, bk_t, kts, 0, hp, True))
        for st in range(4, NKT):
            feeder.add(("v", st), proj_v_gen(st))
        for hp in range(MT):
            feeder.add(("q1", hp),
                       proj_qk_gen(wqsl, bq_t, qts, 1, hp, False))
            feeder.add(("k1", hp),
                       proj_qk_gen(wksl, bk_t, kts, 1, hp, False))

        # half-0 attention
        for hp in range(MT):
            if hp:
                feeder.drain_until(("v", 3))
                feeder.drain_until(("k0", hp))
            attn_core(0, hp, wstack0, feeder.step)
        feeder.add(("n0",), norm_burst_gen(0, wstack0))
        for sub in range(4):
            feeder.add(("op0", sub), outproj_gen(0, wstack0, sub))

        # half-1 attention; per-head norms and out-proj pass A go through
        # the feeder (keeps psflow pool usage strictly sequential)
        feeder.drain_until(("v", NKT - 1))
        feeder.drain_until(("k1", 0))
        for hp in range(MT):
            if hp:
                feeder.drain_until(("k1", hp))
            attn_core(1, hp, wstack1, feeder.step)
            if hp >= 1:
                feeder.add(("n1", hp - 1),
                           norm_one_gen(1, hp - 1, wstack1))
            if hp == 4:
                for sub in range(4):
                    feeder.add(("op1a", sub),
                               op1_passA_gen(wstack1, sub))
        feeder.drain()

        # tail: last head norm, then out-proj pass B
        norm_one(1, 5, wstack1)
        for sub in range(4):
            op1_passB(wstack1, sub)
    return nc


_NC_CACHE = None
LAST_EXEC_NS = None


def _get_nc():
    global _NC_CACHE
    if _NC_CACHE is None:
        nc = build_nc()
        # populate .instr bytes for extended-inst ISA subclasses
        # (the custom-DVE reciprocal) — raw bass skips this pass
        from concourse.library_overlay import lower_extended_insts

        lower_extended_insts(nc)
        _NC_CACHE = nc
    return _NC_CACHE


def kernel(
    normalized_resid_pre, W_Q, W_K, W_V, W_O, b_Q, b_K, b_V, b_O
) -> np.ndarray:
    global LAST_EXEC_NS
    bf = ml_dtypes.bfloat16
    x = np.asarray(normalized_resid_pre, np.float32)
    xT = np.ascontiguousarray(x.transpose(0, 2, 1)).astype(bf)  # [b, D, S]
    wq = np.asarray(W_Q, np.float32).transpose(1, 0, 2).reshape(D, D).astype(bf)
    wk = np.asarray(W_K, np.float32).transpose(1, 0, 2).reshape(D, D).astype(bf)
    wv = np.asarray(W_V, np.float32).transpose(1, 0, 2).reshape(D, D).astype(bf)
    wo = np.asarray(W_O, np.float32).reshape(D, D).astype(bf)
    bq = np.asarray(b_Q, np.float32).reshape(D).copy()
    bk = np.asarray(b_K, np.float32).reshape(D).copy()
    bv = np.asarray(b_V, np.float32).reshape(D).copy()
    bo = np.asarray(b_O, np.float32).reshape(D).copy()

    nc = _get_nc()
    in_maps = [
        {
            "xT": xT[i],
            "wq": wq, "wk": wk, "wv": wv, "wo": wo,
            "bq": bq, "bk": bk, "bv": bv, "bo": bo,
        }
        for i in range(N_CORES)
    ]
    trace = os.environ.get("KERNEL_TRACE", "0") == "1"
    res = run_bass_kernel_spmd(
        nc, in_maps, list(range(N_CORES)), trace=trace
    )
    LAST_EXEC_NS = res.exec_time_ns
    out = np.stack(
        [res.results[i]["y"].astype(np.float32) for i in range(N_CORES)], axis=0
    )
    return out


# revision 50
# speedup vs baseline: 1.1367x; 1.1367x over previous
"""Causal multi-head attention block (b=8, s=1024, d_model=768, 12 heads x 64)
on 8 TRN2 NeuronCores — batch-parallel: core i computes batch element i.

Self-contained: includes the NTFF-profile-hook shim and the BIR wait-split
workaround for this walrus build (max 1 semaphore wait per instruction).

Per-core plan (bf16 matmuls, fp32 PSUM accumulation):
  A. x arrives bf16 (host-transposed) -> xT tiles [m][128,1024] via HWDGE
     (sync/scalar rings), weights ordered by first use (wv, wq, wk early;
     wo late on the gpsimd SWDGE ring).
  B. QT/KT [hd-blk][128,1024] = W.T @ xT (head-pair packed); V in natural
     [s,hd] layout padded to 65 cols/head with a ones column (rowsum trick).
     Only the half-0 slices + V st0..3 are emitted up front; everything
     else drains through a feeder into PE gaps inside the attention loop.
  C. attention is software-pipelined per q-half(512)/head-pair:
     scores(kt+1) is emitted before pv(kt) so the PE never waits on the
     exp of the current tile; exp on ACT (1/8 scale folded), causal mask
     via gpsimd affine_select on diagonal k-tiles, PV accumulates
     [65,512]x2 in PSUM (row 64 = softmax denominator).
  D. denominators collect into a [6,1024] tile per half; one DVE
     reciprocal_approx_fast + f32r copy per half; per-head broadcast via
     K=1 matmul; normalize into the [hd,q] stack; out-proj + b_O; DMA out.
"""

import os
import sys
import types

import numpy as np

# ---------------------------------------------------------------------------
# environment shims


def _install_ntff_hook():
    try:
        import antenv
        from trn_agent_boot.trn_boot import _ntff_profile_via_ctypes
    except Exception:
        return
    if "antenv.axon_hooks" in sys.modules:
        return
    hook = _ntff_profile_via_ctypes("/opt/axon/libaxon_pjrt.so")
    m = types.ModuleType("antenv.axon_hooks")
    m.set_axon_ntff_profile_hook = lambda h: None
    m.get_axon_ntff_profile_hook = lambda: hook
    sys.modules["antenv.axon_hooks"] = m
    antenv.axon_hooks = m


def _install_waitsplit(max_waits=1):
    """walrus on this build rejects >1 sem wait per instruction; split extras
    onto preceding NoOps (same engine, program order preserved)."""
    import json

    import concourse.bass as bass

    if getattr(bass.Bass, "_waitsplit_installed", False):
        return
    counter = [0]

    def _split(inst):
        si = inst.get("sync_info")
        if not si:
            return [inst]
        waits = si.get("on_wait") or []
        if len(waits) <= max_waits:
            return [inst]
        out = []
        head, rest = waits[:-max_waits], waits[-max_waits:]
        for i in range(0, len(head), max_waits):
            counter[0] += 1
            out.append(
                {
                    "debug": inst.get("debug", 0),
                    "engine": inst["engine"],
                    "ins": [],
                    "name": f"I-waitsplit-{counter[0]}",
                    "opcode": "NoOp",
                    "outs": [],
                    "text_hint": "waitsplit",
                    "sync_info": {
                        "on_update": [],
                        "on_wait": head[i : i + max_waits],
                    },
                }
            )
        si["on_wait"] = rest
        out.append(inst)
        return out

    orig = bass.Bass.to_json_bytes

    def to_json_bytes(self):
        d = json.loads(orig(self))
        changed = False
        for f in d.get("functions", []):
            for bb in f.get("blocks", []):
                new = []
                for inst in bb.get("instructions", []):
                    parts = _split(inst)
                    changed = changed or len(parts) > 1
                    new.extend(parts)
                bb["instructions"] = new
        return json.dumps(d).encode() if changed else orig(self)

    bass.Bass.to_json_bytes = to_json_bytes
    bass.Bass._waitsplit_installed = True


_install_ntff_hook()
_install_waitsplit()

import ml_dtypes  # noqa: E402
import concourse.bass as bass  # noqa: E402
import concourse.mybir as mybir  # noqa: E402
import concourse.tile as tile  # noqa: E402
from concourse.bass_utils import run_bass_kernel_spmd  # noqa: E402

# ---------------------------------------------------------------------------
# problem constants (hardcoded per harness contract)

B, S, D, H, DH = 8, 1024, 768, 12, 64
P = 128
MT = D // P            # 6 tiles over d_model / hd
QC = 256               # q-chunk width
QH = 512               # q-half (pair of chunks)
NKT = S // P           # 8 k-tiles over seq
SCALE = float(1.0 / np.sqrt(DH))
N_CORES = 8

F32 = mybir.dt.float32
F32R = mybir.dt.float32r
BF16 = mybir.dt.bfloat16
MMDT = BF16


def build_nc() -> bass.Bass:
    nc = bass.Bass()
    xT = nc.declare_dram_parameter("xT", [D, S], MMDT, isOutput=False)
    wq = nc.declare_dram_parameter("wq", [D, D], MMDT, isOutput=False)
    wk = nc.declare_dram_parameter("wk", [D, D], MMDT, isOutput=False)
    wv = nc.declare_dram_parameter("wv", [D, D], MMDT, isOutput=False)
    wo = nc.declare_dram_parameter("wo", [D, D], MMDT, isOutput=False)
    bq = nc.declare_dram_parameter("bq", [D], F32, isOutput=False)
    bk = nc.declare_dram_parameter("bk", [D], F32, isOutput=False)
    bv = nc.declare_dram_parameter("bv", [D], F32, isOutput=False)
    bo = nc.declare_dram_parameter("bo", [D], F32, isOutput=False)
    y = nc.declare_dram_parameter("y", [S, D], F32, isOutput=True)

    Exp = mybir.ActivationFunctionType.Exp
    Ident = mybir.ActivationFunctionType.Identity
    mult = mybir.AluOpType.mult
    add = mybir.AluOpType.add
    is_ge = mybir.AluOpType.is_ge

    from collections import deque
    from contextlib import ExitStack

    with ExitStack() as _ctx:
        tc = _ctx.enter_context(tile.TileContext(nc))
        constp = _ctx.enter_context(tc.tile_pool(name="const", bufs=1))
        xtp = _ctx.enter_context(tc.tile_pool(name="xT", bufs=1))
        qtp = _ctx.enter_context(tc.tile_pool(name="qt", bufs=1))
        ktp = _ctx.enter_context(tc.tile_pool(name="kt", bufs=1))
        vpp = _ctx.enter_context(tc.tile_pool(name="vp", bufs=1))
        wtsp = _ctx.enter_context(tc.tile_pool(name="wts", bufs=24))
        expp = _ctx.enter_context(tc.tile_pool(name="expst", bufs=6))
        wsp = _ctx.enter_context(tc.tile_pool(name="wstack", bufs=12))
        outp = _ctx.enter_context(tc.tile_pool(name="outsb", bufs=2))
        nrmp = _ctx.enter_context(tc.tile_pool(name="nrm", bufs=2))
        psflow = _ctx.enter_context(
            tc.tile_pool(name="ps_flow", bufs=2, space="PSUM")
        )
        psacc = _ctx.enter_context(
            tc.tile_pool(name="ps_acc", bufs=2, space="PSUM")
        )
        scpp = _ctx.enter_context(
            tc.tile_pool(name="ps_scp", bufs=2, space="PSUM")
        )

        # ---- input DMAs, ordered by first use --------------------------
        # HWDGE rings: sync (SP) and scalar (ACT); SWDGE (gpsimd) for the
        # late-needed W_O and small rows.
        def load_sliced(dram, cols, pool, tag, engines):
            tiles = []
            for mt in range(MT):
                t = pool.tile([P, cols], MMDT, tag=f"{tag}{mt}", bufs=1,
                              name=f"{tag}{mt}")
                engines[mt % len(engines)].dma_start(
                    t[:], dram[mt * P : (mt + 1) * P, :]
                )
                tiles.append(t)

            def sl(mt, a, b):
                return tiles[mt][:, a:b]
            return sl

        hw = [nc.sync, nc.scalar]
        xsl = load_sliced(xT, S, xtp, "xT", hw)
        bq_t = constp.tile([P, MT], F32, tag="bq")  # col hdb = bias block
        bk_t = constp.tile([P, MT], F32, tag="bk")
        nc.sync.dma_start(bq_t[:], bq.rearrange("(c p) -> p c", p=P))
        nc.scalar.dma_start(bk_t[:], bk.rearrange("(c p) -> p c", p=P))
        bv_stage = constp.tile([1, D], F32, tag="bvstage")
        bo_stage = constp.tile([1, D], F32, tag="bostage")
        nc.gpsimd.dma_start(bv_stage[:], bv.rearrange("(o d) -> o d", o=1))
        nc.gpsimd.dma_start(bo_stage[:], bo.rearrange("(o d) -> o d", o=1))
        wqsl = load_sliced(wq, D, wtsp, "wq", hw)
        wksl = load_sliced(wk, D, wtsp, "wk", hw)
        wvsl = load_sliced(wv, D, wtsp, "wv", hw)
        wosl = load_sliced(wo, D, wtsp, "wo", [nc.gpsimd])

        # ---- on-chip constants ----------------------------------------
        ones_stage = constp.tile([1, P], F32, tag="onesstage")
        nc.vector.memset(ones_stage[:], 1.0)
        ones_row = constp.tile([1, P], F32R, tag="onesrow")
        nc.vector.tensor_copy(ones_row[:], ones_stage[:])
        ones_col = constp.tile([P, H], F32, tag="onescol")
        nc.vector.memset(ones_col[:], 1.0)
        bv_row = constp.tile([1, D], F32R, tag="bvrow")
        bo_row = constp.tile([1, D], F32R, tag="borow")
        nc.vector.tensor_copy(bv_row[:], bv_stage[:])
        nc.vector.tensor_copy(bo_row[:], bo_stage[:])

        # selector lhsT for the denominator broadcast: one K=33 matmul
        # replicates row 0 over out partitions 0-63 and row 32 over 64-127
        sel2 = constp.tile([33, P], BF16, tag="sel2")
        nc.vector.memset(sel2[:], 0.0)
        nc.vector.memset(sel2[0:1, 0:64], 1.0)
        nc.vector.memset(sel2[32:33, 64:128], 1.0)
        # per-(half,head) softmax denominator rows: sub0 at partition 0,
        # sub1 at partition 32; rows 1-31 zeroed once (0 x garbage = NaN)
        dns = [
            [
                constp.tile([33, QH], BF16, tag=f"dn{pp}_{hp}",
                            name=f"dn{pp}_{hp}")
                for hp in range(MT)
            ]
            for pp in range(2)
        ]
        for pp in range(2):
            for hp in range(MT):
                nc.vector.memset(dns[pp][hp][:], 0.0)

        # broadcast bias rows to all partitions via K=1 outer-product matmul
        # (emitted after the prologue projections so they don't block PE on
        # the bv/bo DMAs — see emission order below)
        bv_b = constp.tile([P, D], F32, tag="bvb")
        bo_b = constp.tile([P, D], F32, tag="bob")

        def emit_bias_broadcasts():
            for row, bcast in ((bv_row, bv_b), (bo_row, bo_b)):
                for c0, c1 in ((0, 512), (512, 768)):
                    bps = psflow.tile([P, 512], F32, tag="ps", name="bps")
                    nc.tensor.matmul(
                        bps[:, : c1 - c0],
                        ones_row[:],
                        row[:, c0:c1],
                        start=True,
                        stop=True,
                        skip_group_check=True,
                    )
                    nc.vector.tensor_copy(bcast[:, c0:c1], bps[:, : c1 - c0])

        # ---- projection generators (feeder work units) -----------------
        qts = [qtp.tile([P, S], MMDT, tag=f"qt{i}", name=f"qt{i}") for i in range(MT)]
        kts = [ktp.tile([P, S], MMDT, tag=f"kt{i}", name=f"kt{i}") for i in range(MT)]
        vps = [
            vpp.tile([P, H * 65], MMDT, tag=f"vp{st}", name=f"vp{st}")
            for st in range(NKT)
        ]

        def proj_qk_gen(wsl, b_t, dst, sc, hdb, on_act):
            s0 = sc * 512
            ps0 = psflow.tile([P, 512], F32, tag="ps", name="pj0")
            for mt in range(MT):
                nc.tensor.matmul(
                    ps0[:], wsl(mt, hdb * P, (hdb + 1) * P),
                    xsl(mt, s0, s0 + 512),
                    start=(mt == 0), stop=(mt == MT - 1),
                    skip_group_check=True,
                )
                if mt == 2:
                    yield
            if on_act:
                # fused bias add on the scalar engine (per-partition bias)
                nc.scalar.activation(
                    dst[hdb][:, s0 : s0 + 512], ps0[:], Ident,
                    bias=b_t[:, hdb : hdb + 1],
                )
            else:
                bsl = b_t[:, hdb : hdb + 1]
                bb = bass.AP(bsl.tensor, bsl.offset, [bsl.ap[0], [0, 512]])
                nc.vector.tensor_tensor(
                    dst[hdb][:, s0 : s0 + 512], ps0[:], bb, op=add
                )
            yield

        def proj_qk_piece(w_t, b_t, dst, sc, hdb, on_act=False):
            for _ in proj_qk_gen(w_t, b_t, dst, sc, hdb, on_act):
                pass

        def proj_v_gen(st, atomic=False):
            vv = vps[st].rearrange("p (h c) -> p h c", c=65)
            nc.vector.tensor_copy(
                vv[:, :, 64:65],
                ones_col.rearrange("p (h c) -> p h c", c=1),
            )
            ps0 = psflow.tile([P, 512], F32, tag="ps", name="pv0")
            ps1 = psflow.tile([P, 512], F32, tag="ps", name="pv1")
            for mt in range(MT):
                lx = xsl(mt, st * P, (st + 1) * P)
                nc.tensor.matmul(
                    ps0[:], lx, wvsl(mt, 0, 512),
                    start=(mt == 0), stop=(mt == MT - 1),
                    skip_group_check=True,
                )
                nc.tensor.matmul(
                    ps1[:, 0:256], lx, wvsl(mt, 512, 768),
                    start=(mt == 0), stop=(mt == MT - 1),
                    skip_group_check=True,
                )
                if not atomic and mt in (1, 3):
                    yield
            bsrc = bv_b.rearrange("p (h c) -> p h c", c=DH)
            nc.vector.tensor_tensor(
                vv[:, 0:8, 0:DH],
                ps0.rearrange("p (h c) -> p h c", c=DH),
                bsrc[:, 0:8, :],
                op=add,
            )
            nc.vector.tensor_tensor(
                vv[:, 8:12, 0:DH],
                ps1[:, 0:256].rearrange("p (h c) -> p h c", c=DH),
                bsrc[:, 8:12, :],
                op=add,
            )
            yield

        def proj_v(st):
            for _ in proj_v_gen(st):
                pass

        def outproj_gen(pp, wstack, sub):
            q0 = pp * QH
            opsa = psflow.tile([P, 512], F32, tag="ps", name="opa_t")
            opsb = psflow.tile([P, 512], F32, tag="ps", name="opb_t")
            for hdt in range(MT):
                lw = wstack[hdt][:, sub * P : (sub + 1) * P]
                nc.tensor.matmul(
                    opsa[:], lw, wosl(hdt, 0, 512),
                    start=(hdt == 0), stop=(hdt == MT - 1),
                    skip_group_check=True,
                )
                nc.tensor.matmul(
                    opsb[:, 0:256], lw, wosl(hdt, 512, 768),
                    start=(hdt == 0), stop=(hdt == MT - 1),
                    skip_group_check=True,
                )
                if hdt in (1, 3):
                    yield
            osb = outp.tile([P, D], F32, tag="osb")
            nc.vector.tensor_tensor(
                osb[:, 0:512], opsa[:], bo_b[:, 0:512], op=add
            )
            nc.vector.tensor_tensor(
                osb[:, 512:768], opsb[:, 0:256], bo_b[:, 512:768], op=add
            )
            nc.sync.dma_start(
                y[q0 + sub * P : q0 + (sub + 1) * P, :], osb[:]
            )
            yield

        def outproj_sub(pp, wstack, sub):
            for _ in outproj_gen(pp, wstack, sub):
                pass

        # half-1 out-projection in two passes: pass A (heads 0-3 + b_O)
        # stages to SBUF and is feedable during the last attention head;
        # pass B (heads 4-5) merges and writes out — the only true tail.
        stA = [
            nrmp.tile([P, D], F32, tag=f"stA{s}", bufs=1, name=f"stA{s}")
            for s in range(4)
        ]

        def op1_passA_gen(wstack, sub):
            opsa = psflow.tile([P, 512], F32, tag="ps", name="opa_t")
            opsb = psflow.tile([P, 512], F32, tag="ps", name="opb_t")
            for hdt in range(4):
                lw = wstack[hdt][:, sub * P : (sub + 1) * P]
                nc.tensor.matmul(
                    opsa[:], lw, wosl(hdt, 0, 512),
                    start=(hdt == 0), stop=(hdt == 3),
                    skip_group_check=True,
                )
                nc.tensor.matmul(
                    opsb[:, 0:256], lw, wosl(hdt, 512, 768),
                    start=(hdt == 0), stop=(hdt == 3),
                    skip_group_check=True,
                )
                if hdt == 1:
                    yield
            nc.vector.tensor_tensor(
                stA[sub][:, 0:512], opsa[:], bo_b[:, 0:512], op=add
            )
            nc.vector.tensor_tensor(
                stA[sub][:, 512:768], opsb[:, 0:256], bo_b[:, 512:768], op=add
            )
            yield

        def op1_passB(wstack, sub):
            opsa = psflow.tile([P, 512], F32, tag="ps", name="opa_t")
            opsb = psflow.tile([P, 512], F32, tag="ps", name="opb_t")
            for hdt in range(4, MT):
                lw = wstack[hdt][:, sub * P : (sub + 1) * P]
                nc.tensor.matmul(
                    opsa[:], lw, wosl(hdt, 0, 512),
                    start=(hdt == 4), stop=(hdt == MT - 1),
                    skip_group_check=True,
                )
                nc.tensor.matmul(
                    opsb[:, 0:256], lw, wosl(hdt, 512, 768),
                    start=(hdt == 4), stop=(hdt == MT - 1),
                    skip_group_check=True,
                )
            osb = outp.tile([P, D], F32, tag="osb")
            nc.vector.tensor_tensor(
                osb[:, 0:512], opsa[:], stA[sub][:, 0:512], op=add
            )
            nc.vector.tensor_tensor(
                osb[:, 512:768], opsb[:, 0:256], stA[sub][:, 512:768], op=add
            )
            nc.sync.dma_start(
                y[QH + sub * P : QH + (sub + 1) * P, :], osb[:]
            )

        def norm_one(pp, hp, wstack):
            """Broadcast the head-pair's two raw denominator rows into one
            [128,512] PSUM tile via a single K=33 selector matmul,
            reciprocal across all 128 partitions at once, then one
            full-width normalize multiply."""
            rb = psflow.tile([P, 512], F32, tag="ps", name="rb")
            nc.tensor.matmul(
                rb[:], sel2[:], dns[pp][hp][:],
                start=True, stop=True, skip_group_check=True,
            )
            rsb = nrmp.tile([P, 512], F32, tag="rsb", name="rsb")
            nc.vector.reciprocal_approx_fast(rsb[:], rb[:])
            nc.vector.tensor_tensor(
                wstack[hp][:], wstack[hp][:], rsb[:], op=mult,
            )

        def norm_burst_gen(pp, wstack):
            for hp in range(MT):
                norm_one(pp, hp, wstack)
                yield

        def norm_one_gen(pp, hp, wstack):
            norm_one(pp, hp, wstack)
            yield

        class Feeder:
            """Doles out deferred emission work in ~2-3-matmul steps so the
            PE stream interleaves finely with attention matmuls."""

            def __init__(self):
                self.q = deque()

            def add(self, tag, gen):
                self.q.append((tag, gen))

            def step(self):
                while self.q:
                    try:
                        next(self.q[0][1])
                        return
                    except StopIteration:
                        self.q.popleft()

            def drain_until(self, tag):
                while any(t == tag for t, _ in self.q):
                    try:
                        next(self.q[0][1])
                    except StopIteration:
                        self.q.popleft()

            def drain(self):
                while self.q:
                    self.step()

        feeder = Feeder()

        def attn_core(pp, hp, wstack, feed):
            q0 = pp * QH
            nkt0 = 4 * pp + 2
            nkt1 = 4 * pp + 4
            pvs = [
                psacc.tile([65, QH], F32, tag="pv", name=f"pv{sub}")
                for sub in range(2)
            ]
            scps = {}
            ests = {}

            def emit_scores(kt):
                c0 = 0 if kt < nkt0 else QC
                scp = scpp.tile([P, 2 * QH], F32, tag="scp", name="scp")
                for sub in range(2):
                    r0 = sub * 64
                    nc.tensor.matmul(
                        scp[:, sub * QH + c0 : (sub + 1) * QH],
                        kts[hp][r0 : r0 + 64, kt * P : (kt + 1) * P],
                        qts[hp][r0 : r0 + 64, q0 + c0 : q0 + QH],
                        start=True,
                        stop=True,
                        tile_position=(r0, 0),
                        skip_group_check=True,
                    )
                scps[kt] = (scp, c0)

            def emit_exp_mask(kt):
                scp, c0 = scps.pop(kt)
                w = QH - c0
                est = expp.tile([P, 2 * QH], MMDT, tag="est", name="est")
                if c0 == 0:
                    nc.scalar.activation(est[:], scp[:], Exp, scale=SCALE)
                else:
                    sin = bass.AP(
                        scp.tensor, scp.offset + c0,
                        [scp.ap[0], [QH, 2], [1, w]],
                    )
                    sout = bass.AP(
                        est.tensor, est.offset + c0,
                        [est.ap[0], [QH, 2], [1, w]],
                    )
                    nc.scalar.activation(sout, sin, Exp, scale=SCALE)
                for sub in range(2):
                    b0 = sub * QH
                    if kt in (4 * pp, 4 * pp + 1):
                        nc.gpsimd.affine_select(
                            est[:, b0 : b0 + QC], est[:, b0 : b0 + QC],
                            pattern=[[1, QC]],
                            compare_op=is_ge, fill=0.0,
                            base=(0 if kt == 4 * pp else -P),
                            channel_multiplier=-1,
                        )
                    if kt in (4 * pp + 2, 4 * pp + 3):
                        nc.gpsimd.affine_select(
                            est[:, b0 + QC : b0 + QH],
                            est[:, b0 + QC : b0 + QH],
                            pattern=[[1, QC]],
                            compare_op=is_ge, fill=0.0,
                            base=(0 if kt == 4 * pp + 2 else -P),
                            channel_multiplier=-1,
                        )
                ests[kt] = (est, c0)

            def emit_pv(kt):
                est, c0 = ests.pop(kt)
                for sub in range(2):
                    h = 2 * hp + sub
                    nc.tensor.matmul(
                        pvs[sub][:, c0:QH],
                        vps[kt][:, h * 65 : (h + 1) * 65],
                        est[:, sub * QH + c0 : (sub + 1) * QH],
                        start=(kt == 0),
                        stop=(kt == nkt1 - 1),
                        skip_group_check=True,
                    )

            emit_scores(0)
            for kt in range(nkt1):
                if kt + 1 < nkt1:
                    emit_scores(kt + 1)
                emit_exp_mask(kt)
                feed()
                emit_pv(kt)
            # stash frees the PV banks: unnormalized rows into wstack (bf16;
            # half-0 casts ride the scalar engine, which has slack there),
            # denominator rows into partitions 0/32 of the dn tile
            for sub in range(2):
                r0 = sub * 64
                if pp == 0:
                    nc.scalar.activation(
                        wstack[hp][r0 : r0 + 64, :], pvs[sub][0:64, :], Ident
                    )
                else:
                    nc.vector.tensor_copy(
                        wstack[hp][r0 : r0 + 64, :], pvs[sub][0:64, :]
                    )
                nc.vector.tensor_copy(
                    dns[pp][hp][32 * sub : 32 * sub + 1, :],
                    pvs[sub][64:65, :],
                )

        # ---- emission order -------------------------------------------
        wstack0 = [
            wsp.tile([P, QH], MMDT, tag="ws", name=f"ws0_{i}")
            for i in range(MT)
        ]
        wstack1 = [
            wsp.tile([P, QH], MMDT, tag="ws", name=f"ws1_{i}")
            for i in range(MT)
        ]

        # prologue: bias broadcasts (bv_b must be written before proj_v's
        # epilogue reads it), Q/K half-0 hp0 (wq/wk arrive first on the
        # HWDGE rings), V st0; V st1-3 are atomic feeder chunks consumed
        # inside hp0's kt loop just ahead of their pv(kt) consumers.
        emit_bias_broadcasts()
        proj_qk_piece(wqsl, bq_t, qts, 0, 0, on_act=True)
        proj_qk_piece(wksl, bk_t, kts, 0, 0, on_act=True)
        proj_v(0)

        for st in range(1, 4):
            feeder.add(("v", st), proj_v_gen(st, atomic=True))
        for hp in range(1, MT):
            feeder.add(("q0", hp),
                       proj_qk_gen(wqsl, bq_t, qts, 0, hp, True))
            feeder.add(("k0", hp),
                       proj_qk_gen(wksl, bk_t, kts, 0, hp, True))
        for st in range(4, NKT):
            feeder.add(("v", st), proj_v_gen(st))
        for hp in range(MT):
            feeder.add(("q1", hp),
                       proj_qk_gen(wqsl, bq_t, qts, 1, hp, False))
            feeder.add(("k1", hp),
                       proj_qk_gen(wksl, bk_t, kts, 1, hp, False))

        # half-0 attention
        for hp in range(MT):
            if hp:
                feeder.drain_until(("v", 3))
                feeder.drain_until(("k0", hp))
            attn_core(0, hp, wstack0, feeder.step)
        feeder.add(("n0",), norm_burst_gen(0, wstack0))
        for sub in range(4):
            feeder.add(("op0", sub), outproj_gen(0, wstack0, sub))

        # half-1 attention; per-head norms and out-proj pass A go through
        # the feeder (keeps psflow pool usage strictly sequential)
        feeder.drain_until(("v", NKT - 1))
        feeder.drain_until(("k1", 0))
        for hp in range(MT):
            if hp:
                feeder.drain_until(("k1", hp))
            attn_core(1, hp, wstack1, feeder.step)
            if hp >= 1:
                feeder.add(("n1", hp - 1),
                           norm_one_gen(1, hp - 1, wstack1))
            if hp == 4:
                for sub in range(4):
                    feeder.add(("op1a", sub),
                               op1_passA_gen(wstack1, sub))
        feeder.drain()

        # tail: last head norm, then out-proj pass B
        norm_one(1, 5, wstack1)
        for sub in range(4):
            op1_passB(wstack1, sub)
    return nc


_NC_CACHE = None
LAST_EXEC_NS = None


def _get_nc():
    global _NC_CACHE
    if _NC_CACHE is None:
        nc = build_nc()
        # populate .instr bytes for extended-inst ISA subclasses
        # (the custom-DVE reciprocal) — raw bass skips this pass
        from concourse.library_overlay import lower_extended_insts

        lower_extended_insts(nc)
        _NC_CACHE = nc
    return _NC_CACHE


def kernel(
    normalized_resid_pre, W_Q, W_K, W_V, W_O, b_Q, b_K, b_V, b_O
) -> np.ndarray:
    global LAST_EXEC_NS
    bf = ml_dtypes.bfloat16
    x = np.asarray(normalized_resid_pre, np.float32)
    xT = np.ascontiguousarray(x.transpose(0, 2, 1)).astype(bf)  # [b, D, S]
    wq = np.asarray(W_Q, np.float32).transpose(1, 0, 2).reshape(D, D).astype(bf)
    wk = np.asarray(W_K, np.float32).transpose(1, 0, 2).reshape(D, D).astype(bf)
    wv = np.asarray(W_V, np.float32).transpose(1, 0, 2).reshape(D, D).astype(bf)
    wo = np.asarray(W_O, np.float32).reshape(D, D).astype(bf)
    bq = np.asarray(b_Q, np.float32).reshape(D).copy()
    bk = np.asarray(b_K, np.float32).reshape(D).copy()
    bv = np.asarray(b_V, np.float32).reshape(D).copy()
    bo = np.asarray(b_O, np.float32).reshape(D).copy()

    nc = _get_nc()
    in_maps = [
        {
            "xT": xT[i],
            "wq": wq, "wk": wk, "wv": wv, "wo": wo,
            "bq": bq, "bk": bk, "bv": bv, "bo": bo,
        }
        for i in range(N_CORES)
    ]
    trace = os.environ.get("KERNEL_TRACE", "0") == "1"
    res = run_bass_kernel_spmd(
        nc, in_maps, list(range(N_CORES)), trace=trace
    )
    LAST_EXEC_NS = res.exec_time_ns
    out = np.stack(
        [res.results[i]["y"].astype(np.float32) for i in range(N_CORES)], axis=0
    )
    return out
